# revision 1
# baseline (speedup 1.0000x reference)
"""Trainium2 Bass kernel for nn_CorrelationBranch.

Computes, per batch b:
    v   = W @ x_vit[b] + bias          (1x1 conv projection, 512 -> 256)
    vup = bilinear_upsample4x(v)       (32x32 -> 128x128, half-pixel centers)
    sim = cossim_over_channels(x_cnn[b], vup)   -> [1, 128, 128]

Sharding: data-parallel over batch, one batch per NeuronCore (8 cores).

Per-core layout strategy:
  - projection on PE: v0T[hw, o] = sum_c x_vit[c, hw] * Wt[c, o]  (f32r matmuls)
    produced as 31 overlapping "h-pair" tiles (64 partitions = 2 h-rows x 32 w).
  - bilinear upsample on PE: for each output row I, one matmul
      vup_tile[J, o] = U_I.T @ v0_pair[h0(I)]   with U_I a constant 64x128
      bilinear weight matrix (6 distinct alpha classes).
  - x_cnn tiles transposed to pixel-major on PE (fp32 transpose via identity).
  - reductions on DVE scalar_tensor_tensor with accum_out (free-dim = channels):
      dot = sum_c a*v, na2 = sum_c a*a, nv2 = sum_c v*v
  - final: sim = dot / (sqrt(na2*nv2) + 1e-8) on DVE/ACT, output transposed
    on host (kernel emits [J, I]).
"""

import numpy as np

import concourse.bacc as bacc
import concourse.mybir as mybir
import concourse.tile as tile
from concourse.bass_utils import run_bass_kernel_spmd

F32 = mybir.dt.float32
F32R = mybir.dt.float32r
Alu = mybir.AluOpType
Act = mybir.ActivationFunctionType

B, C, H, W_ = 8, 256, 128, 128
V, h_, w_ = 512, 32, 32
HW_LO = h_ * w_          # 1024
HW_HI = H * W_           # 16384
N_PAIR = 31              # h-pair tiles (p, p+1) for p in 0..30
EPS = 1e-8


def _lin_axis_weights(n_out, n_in):
    """Per output index: clamped half-pixel bilinear weights over n_in inputs."""
    Wm = np.zeros((n_in, n_out), np.float64)
    for j in range(n_out):
        src = (j + 0.5) * n_in / n_out - 0.5
        w0 = int(np.floor(src))
        f = src - w0
        for idx, wt in ((w0, 1.0 - f), (w0 + 1, f)):
            Wm[min(max(idx, 0), n_in - 1), j] += wt
    return Wm


def make_upsample_consts():
    """Returns (U_all [n_cls, 64, 128] fp32, alpha_idx [128], pair_idx [128])."""
    Bw = _lin_axis_weights(W_, w_)            # (32, 128) J-axis weights
    Ah = _lin_axis_weights(H, h_)             # (32, 128) I-axis weights (cols=I)
    classes = {}
    alpha_idx = np.zeros(H, np.int64)
    pair_idx = np.zeros(H, np.int64)
    for I in range(H):
        col = Ah[:, I]
        nz = np.nonzero(col)[0]
        p = min(int(nz.min()), h_ - 2)
        a0, a1 = col[p], col[p + 1]
        key = (float(a0), float(a1))
        if key not in classes:
            classes[key] = len(classes)
        alpha_idx[I] = classes[key]
        pair_idx[I] = p
    U_all = np.zeros((len(classes), 2 * w_, W_), np.float32)
    for (a0, a1), cls in classes.items():
        U_all[cls, :w_, :] = (a0 * Bw).astype(np.float32)
        U_all[cls, w_:, :] = (a1 * Bw).astype(np.float32)
    return U_all, alpha_idx, pair_idx


U_ALL, ALPHA_IDX, PAIR_IDX = make_upsample_consts()
N_CLS = U_ALL.shape[0]


def build_nc(repeat=1, ablate=()):
    ablate = set(ablate)
    nc = bacc.Bacc(None, target_bir_lowering=False)

    x_vit_d = nc.dram_tensor("x_vit_b", [V, HW_LO], F32, kind="ExternalInput")
    x_cnn_d = nc.dram_tensor("x_cnn_b", [C, HW_HI], F32, kind="ExternalInput")
    wt_d = nc.dram_tensor("Wt", [V, C], F32, kind="ExternalInput")
    bias_d = nc.dram_tensor("bias", [1, C], F32, kind="ExternalInput")
    u_d = nc.dram_tensor("U_all", [N_CLS, 2 * w_, W_], F32, kind="ExternalInput")
    id_d = nc.dram_tensor("ident", [128, 128], F32, kind="ExternalInput")
    out_d = nc.dram_tensor("simT", [W_, H], F32, kind="ExternalOutput")

    x_cnn_r = x_cnn_d.rearrange("(q c) (i j) -> c q i j", c=128, j=W_)

    with tile.TileContext(nc) as tc:
        with tc.tile_pool(name="consts", bufs=1) as consts:
            xv_f = consts.tile([128, V // 128, HW_LO], F32)
            nc.sync.dma_start(out=xv_f, in_=x_vit_d.rearrange("(q c) f -> c q f", c=128))
            wt_f = consts.tile([128, V // 128, C], F32)
            nc.sync.dma_start(out=wt_f, in_=wt_d.rearrange("(q c) o -> c q o", c=128))
            bias_f = consts.tile([1, C], F32)
            nc.sync.dma_start(out=bias_f, in_=bias_d[:, :])
            u_f = consts.tile([2 * w_, N_CLS, W_], F32)
            nc.sync.dma_start(out=u_f, in_=u_d.rearrange("u k m -> k u m"))
            id_t = consts.tile([128, 128], F32)
            nc.sync.dma_start(out=id_t, in_=id_d[:, :])

            # f32r-rounded copies for PE consumption
            xv_r = consts.tile([128, V // 128, HW_LO], F32R)
            nc.vector.tensor_copy(xv_r, xv_f)
            wt_r = consts.tile([128, V // 128, C], F32R)
            nc.vector.tensor_copy(wt_r, wt_f)
            u_r = consts.tile([2 * w_, N_CLS, W_], F32R)
            nc.vector.tensor_copy(u_r, u_f)
            ones_f = consts.tile([1, 2 * w_], F32)
            nc.vector.memset(ones_f, 1.0)
            ones_r = consts.tile([1, 2 * w_], F32R)
            nc.vector.tensor_copy(ones_r, ones_f)
            bias_r = consts.tile([1, C], F32R)
            nc.vector.tensor_copy(bias_r, bias_f)

            for rep in range(repeat):
                with tc.tile_pool(name="v0sb", bufs=1) as v0pool, \
                     tc.tile_pool(name="maps", bufs=1) as maps:
                    v0sb = v0pool.tile([2 * w_, N_PAIR, C], F32R)
                    dot_map = maps.tile([128, H], F32)
                    na2_map = maps.tile([128, H], F32)
                    nv2_map = maps.tile([128, H], F32)
                    if "stt" in ablate:
                        nc.vector.memset(dot_map, 1.0)
                        nc.vector.memset(na2_map, 1.0)
                        nc.vector.memset(nv2_map, 1.0)

                    # ---- projection: 31 h-pair tiles ----
                    with tc.tile_pool(name="proj_ps", bufs=2, space="PSUM") as proj_ps:
                        for p in range(0 if "proj" in ablate else N_PAIR):
                            pp = proj_ps.tile([2 * w_, C], F32)
                            for q in range(V // 128):
                                nc.tensor.matmul(
                                    pp,
                                    xv_r[:, q, w_ * p: w_ * p + 2 * w_],
                                    wt_r[:, q, :],
                                    start=(q == 0), stop=False)
                            nc.tensor.matmul(pp, ones_r, bias_r,
                                             start=False, stop=True)
                            nc.scalar.copy(v0sb[:, p, :], pp)

                    # ---- main loop over output rows ----
                    with tc.tile_pool(name="aload", bufs=3) as aload, \
                         tc.tile_pool(name="apm", bufs=3) as apm, \
                         tc.tile_pool(name="vsb", bufs=3) as vsb, \
                         tc.tile_pool(name="junk", bufs=1) as junkp, \
                         tc.tile_pool(name="vup_ps", bufs=3, space="PSUM") as vup_ps, \
                         tc.tile_pool(name="aT_ps", bufs=3, space="PSUM") as aT_ps:
                        junk = junkp.tile([128, C], F32)
                        for g in range(H // 8):
                            a_slab = aload.tile([128, 2, 8, W_], F32, tag="a_slab")
                            if "slabdma" not in ablate:
                                nc.sync.dma_start(
                                    out=a_slab,
                                    in_=x_cnn_r[:, :, 8 * g: 8 * g + 8, :])
                            for i_in in range(8):
                                I = 8 * g + i_in

                                a_pm = apm.tile([128, C], F32)
                                v_sb = vsb.tile([128, C], F32)
                                if "upmm" not in ablate:
                                    vp = vup_ps.tile([128, C], F32)
                                    nc.tensor.matmul(
                                        vp, u_r[:, ALPHA_IDX[I], :], v0sb[:, PAIR_IDX[I], :],
                                        start=True, stop=True)
                                    if "vcopy" not in ablate:
                                        nc.scalar.copy(v_sb, vp)

                                if "tr" not in ablate:
                                    tp = aT_ps.tile([128, C], F32)
                                    nc.tensor.transpose(tp[:, 0:128], a_slab[:, 0, i_in, :], id_t)
                                    nc.tensor.transpose(tp[:, 128:256], a_slab[:, 1, i_in, :], id_t)
                                    if "acopy" not in ablate:
                                        nc.scalar.copy(a_pm, tp)

                                if "stt" not in ablate:
                                    nc.vector.scalar_tensor_tensor(
                                        out=junk, in0=a_pm, scalar=1.0, in1=v_sb,
                                        op0=Alu.mult, op1=Alu.mult,
                                        accum_out=dot_map[:, I: I + 1])
                                    nc.vector.scalar_tensor_tensor(
                                        out=junk, in0=a_pm, scalar=1.0, in1=a_pm,
                                        op0=Alu.mult, op1=Alu.mult,
                                        accum_out=na2_map[:, I: I + 1])
                                    nc.vector.scalar_tensor_tensor(
                                        out=junk, in0=v_sb, scalar=1.0, in1=v_sb,
                                        op0=Alu.mult, op1=Alu.mult,
                                        accum_out=nv2_map[:, I: I + 1])

                    # ---- final combine ----
                    with tc.tile_pool(name="fin", bufs=1) as fin:
                        prod = fin.tile([128, H], F32)
                        nc.vector.tensor_mul(prod, na2_map, nv2_map)
                        sq = fin.tile([128, H], F32)
                        nc.scalar.activation(out=sq, in_=prod, func=Act.Sqrt)
                        denom = fin.tile([128, H], F32)
                        nc.vector.tensor_scalar_add(denom, sq, EPS)
                        rec = fin.tile([128, H], F32)
                        nc.vector.reciprocal(rec, denom)
                        simt = fin.tile([128, H], F32)
                        nc.vector.tensor_mul(simt, dot_map, rec)
                        nc.sync.dma_start(out=out_d[:, :], in_=simt)

    nc.compile()
    return nc


import concourse.bass_isa as bass_isa

# ============================ V2: big-op channel-major ============================
# Strategy: this environment charges ~40-100us per instruction, so V2 minimizes
# instruction count: channel-major layout end to end (no transposes of x_cnn),
# bilinear upsample as 4-phase strided DVE ops with edge padding, products as
# full-slab tensor_tensor ops, channel reduction via gpsimd partition_all_reduce,
# per-pixel maps assembled with partition-scatter DMAs (accum over chunks).

# per-phase bilinear constants (identical for H and W axes, scale 4, half-pixel):
# out = c0 * in[b + d] + c1 * in[b + d + 1], with padded index offset +1
PHASES = [(-1, 0.375, 0.625), (-1, 0.125, 0.875), (0, 0.875, 0.125), (0, 0.625, 0.375)]


def build_nc_v2(repeat=1):
    nc = bacc.Bacc(None, target_bir_lowering=False)

    x_vit_d = nc.dram_tensor("x_vit_b", [V, HW_LO], F32, kind="ExternalInput")
    x_cnn_d = nc.dram_tensor("x_cnn_b", [C, HW_HI], F32, kind="ExternalInput")
    wt_d = nc.dram_tensor("Wt", [V, C], F32, kind="ExternalInput")
    bias_d = nc.dram_tensor("bias", [1, C], F32, kind="ExternalInput")
    out_d = nc.dram_tensor("sim", [H, W_], F32, kind="ExternalOutput")

    BF16 = mybir.dt.bfloat16

    with tile.TileContext(nc) as tc:
        with tc.tile_pool(name="consts", bufs=1) as consts:
            xv_r = consts.tile([128, V // 128, HW_LO], F32R)
            wt_r = consts.tile([128, V // 128, C], F32R)
            bias_r = consts.tile([1, C], F32R)
            ones_r = consts.tile([1, 512], F32R)
            map_dot = consts.tile([128, W_], F32)
            map_na2 = consts.tile([128, W_], F32)
            map_nv2 = consts.tile([128, W_], F32)
            with tc.tile_pool(name="stage_in", bufs=1) as stage_in:
                xv_f = stage_in.tile([128, V // 128, HW_LO], F32)
                nc.sync.dma_start(out=xv_f, in_=x_vit_d.rearrange("(q c) f -> c q f", c=128))
                wt_f = stage_in.tile([128, V // 128, C], F32)
                nc.sync.dma_start(out=wt_f, in_=wt_d.rearrange("(q c) o -> c q o", c=128))
                bias_f = stage_in.tile([1, C], F32)
                nc.sync.dma_start(out=bias_f, in_=bias_d[:, :])
                ones_f = stage_in.tile([1, 512], F32)
                nc.vector.memset(ones_f, 1.0)
                nc.vector.tensor_copy(xv_r, xv_f)
                nc.vector.tensor_copy(wt_r, wt_f)
                nc.vector.tensor_copy(bias_r, bias_f)
                nc.vector.tensor_copy(ones_r, ones_f)

            for rep in range(repeat):
                with tc.tile_pool(name="work", bufs=1) as work, \
                     tc.tile_pool(name="tmps", bufs=1) as tmps, \
                     tc.tile_pool(name="apool", bufs=1) as apool, \
                     tc.tile_pool(name="spool", bufs=1) as spool, \
                     tc.tile_pool(name="rpool", bufs=1) as rpool, \
                     tc.tile_pool(name="proj_ps", bufs=2, space="PSUM") as proj_ps:
                    for cc in range(2):
                        # ---- projection for this channel chunk: v0 (128 o, 1024 hw) ----
                        ps = proj_ps.tile([128, HW_LO], F32)
                        for hf in range(2):
                            sl = slice(512 * hf, 512 * hf + 512)
                            for q in range(V // 128):
                                nc.tensor.matmul(
                                    ps[:, sl],
                                    wt_r[:, q, 128 * cc: 128 * cc + 128],
                                    xv_r[:, q, sl],
                                    start=(q == 0), stop=False)
                            nc.tensor.matmul(
                                ps[:, sl], bias_r[0:1, 128 * cc: 128 * cc + 128],
                                ones_r, start=False, stop=True)

                        # ---- v0 padded along w: (128, 32, 34) ----
                        v0p = work.tile([128, h_, w_ + 2], F32, tag="v0p")
                        psv = ps.rearrange("p (hh ww) -> p hh ww", ww=w_)
                        nc.scalar.copy(v0p[:, :, 1: w_ + 1], psv)
                        nc.scalar.copy(v0p[:, :, 0:1], psv[:, :, 0:1])
                        nc.scalar.copy(v0p[:, :, w_ + 1: w_ + 2], psv[:, :, w_ - 1: w_])

                        # ---- stage-w: upsample along w -> v1p (128, 34, 128) bf16 ----
                        v1p = work.tile([128, h_ + 2, W_], BF16, tag="v1p")
                        for jph, (d, c0, c1) in enumerate(PHASES):
                            tw = tmps.tile([128, h_, w_], F32, tag="tw")
                            nc.vector.tensor_scalar_mul(tw, v0p[:, :, d + 1: d + 33], c0)
                            nc.vector.scalar_tensor_tensor(
                                out=v1p[:, 1: h_ + 1, jph::4],
                                in0=v0p[:, :, d + 2: d + 34], scalar=c1, in1=tw,
                                op0=Alu.mult, op1=Alu.add)
                        nc.vector.tensor_copy(v1p[:, 0:1, :], v1p[:, 1:2, :])
                        nc.vector.tensor_copy(v1p[:, h_ + 1: h_ + 2, :], v1p[:, h_: h_ + 1, :])

                        for hh in range(2):
                            # ---- stage-h for I-rows [64*hh, 64*hh+64) -> vup bf16 ----
                            vup = work.tile([128, 64, W_], BF16, tag="vup")
                            for iph, (d, c0, c1) in enumerate(PHASES):
                                r0 = 16 * hh + d + 1
                                th = tmps.tile([128, 16, W_], BF16, tag="th")
                                nc.vector.tensor_scalar_mul(th, v1p[:, r0: r0 + 16, :], c0)
                                nc.vector.scalar_tensor_tensor(
                                    out=vup[:, iph::4, :],
                                    in0=v1p[:, r0 + 1: r0 + 17, :], scalar=c1, in1=th,
                                    op0=Alu.mult, op1=Alu.add)
                            vupf = vup.rearrange("p a b -> p (a b)")

                            a_t = apool.tile([128, HW_HI // 2], F32, tag="a")
                            nc.sync.dma_start(
                                out=a_t,
                                in_=x_cnn_d[128 * cc: 128 * cc + 128,
                                            8192 * hh: 8192 * hh + 8192])

                            for qi, (mp, i0, i1) in enumerate((
                                    (map_dot, a_t, vupf),
                                    (map_na2, a_t, a_t),
                                    (map_nv2, vupf, vupf))):
                                s_t = spool.tile([128, HW_HI // 2], F32, tag="s")
                                nc.vector.tensor_mul(s_t, i0, i1)
                                r_t = rpool.tile([128, HW_HI // 2], F32, tag="r")
                                nc.gpsimd.partition_all_reduce(
                                    r_t, s_t, 128, bass_isa.ReduceOp.add)
                                nc.gpsimd.dma_start(
                                    out=mp[64 * hh: 64 * hh + 64, :],
                                    in_=r_t[0:1, :],
                                    accum_op=(Alu.add if cc == 1 else Alu.bypass))

                    # ---- final combine on (128, 128) maps ----
                    with tc.tile_pool(name="fin", bufs=1) as fin:
                        t1 = fin.tile([128, W_], F32)
                        nc.vector.tensor_mul(t1, map_na2, map_nv2)
                        t2 = fin.tile([128, W_], F32)
                        nc.scalar.activation(out=t2, in_=t1, func=Act.Sqrt)
                        nc.vector.tensor_scalar_add(t1, t2, EPS)
                        nc.vector.reciprocal(t2, t1)
                        nc.vector.tensor_mul(t1, map_dot, t2)
                        nc.sync.dma_start(out=out_d[:, :], in_=t1)

    nc.compile()
    return nc


_CACHED = {}


def _get_nc(repeat=1, version=2):
    key = (version, repeat)
    if key not in _CACHED:
        _CACHED[key] = build_nc_v2(repeat) if version == 2 else build_nc(repeat)
    return _CACHED[key]


def make_in_maps(x_cnn, x_vit, W, b):
    shared = {
        "Wt": np.ascontiguousarray(W.T).astype(np.float32),
        "bias": b.reshape(1, C).astype(np.float32),
        "U_all": U_ALL,
        "ident": np.eye(128, dtype=np.float32),
    }
    in_maps = []
    for core in range(B):
        m = dict(shared)
        m["x_vit_b"] = np.ascontiguousarray(x_vit[core].reshape(V, HW_LO))
        m["x_cnn_b"] = np.ascontiguousarray(x_cnn[core].reshape(C, HW_HI))
        in_maps.append(m)
    return in_maps


def kernel(x_cnn, x_vit, W, b, _repeat=1, _return_res=False, _version=2):
    x_cnn = np.asarray(x_cnn, dtype=np.float32)
    x_vit = np.asarray(x_vit, dtype=np.float32)
    W = np.asarray(W, dtype=np.float32)
    b = np.asarray(b, dtype=np.float32)
    nc = _get_nc(_repeat, _version)
    in_maps = make_in_maps(x_cnn, x_vit, W, b)
    res = run_bass_kernel_spmd(nc, in_maps, core_ids=list(range(B)))
    out = np.empty((B, 1, H, W_), np.float32)
    for core in range(B):
        r = res.results[core]
        out[core, 0] = r["sim"] if "sim" in r else r["simT"].T
    if _return_res:
        return out, res
    return out

